# revision 17
# baseline (speedup 1.0000x reference)
"""Trainium2 Bass kernel for nn_Attention_Layer (B=8, SH=SV=32, DH=D=256, DV=4096).

Math (see reference):
    U_h = h @ U                  (B,SH,D)
    W_v = v @ W                  (B,SV,D)
    f   = tanh(W_v + U_h + b)    (B,SH,SV,D)
    q   = f @ w                  (B,SH,SV,DV)
    e   = exp(q); S = sum_b e; beta = e/S
    u   = sum_sv beta * v        (B,SH,DV)

Sharding: over SH (4 h-positions per core), no collectives.

v4 design:
  - fp8e4m3 DoubleRow for the Wv and q matmuls (W*8, vT, f, w*16 in fp8;
    scale folded into tanh/exp). rel-err ~8.6e-3 vs the 2e-2 gate.
  - 2 DMA queues: W/w/vrep3 on sync (SP); packed consts, vT and the other
    v_rep tiles on gpsimd; all layouts host-rearranged so every DMA is
    contiguous per partition.
  - All small consts (U2/hT2/I/Lsum) ship as ONE packed uint8 DMA, consumed
    through bitcast views.
  - ev = e*v runs per-b on DVE during the exp phase; only gv = ev*r is
    r-gated. u-block deferral depth 1.
  - Iteration widths [1024,1024,1024,512,512]: the last iteration is small
    and its recip/cast/gv/u chain runs in 256-wide all-DVE chunks, so the
    post-last-exp serial tail is short.
"""

import sys

sys.path.insert(0, "/opt/trn_rl_repo")

from contextlib import ExitStack

import ml_dtypes
import numpy as np

import concourse.bass as bass
import concourse.mybir as mybir
import concourse.tile as tile
from concourse import bacc
from concourse.bass_utils import run_bass_kernel_spmd

BF16 = ml_dtypes.bfloat16
F8 = ml_dtypes.float8_e4m3
F32 = np.float32

B, SH, SV, DH, DV, D = 8, 32, 32, 256, 4096, 256
KT_C0 = DV // 128
NCORES = 8
SHL = SH // NCORES  # 4 h-positions per core
ROWS = B * SHL  # 32 output rows per core, index = b*SHL + h
WIDTHS = [1024, 1024, 1024, 512, 512]
COFFS = [0, 1024, 2048, 3072, 3584]
NIT = len(WIDTHS)
W_SCALE = 8.0
SMALL_SCALE = 16.0  # w scale

# packed-consts byte offsets (per partition)
PK_U2 = 0  # 3*256 f32 = 3072B  (kt0, kt1, bias-row-on-p0)
PK_HT = 3072  # 3*32 f32 = 384B
PK_I = 3456  # 128 bf16 = 256B
PK_L = 3712  # 8*32 bf16 = 512B
PK_BYTES = 4224

DR = mybir.MatmulPerfMode.DoubleRow


def build_nc(debug: bool = False):
    nc = bacc.Bacc("TRN2", target_bir_lowering=False, debug=debug)
    f32, bf, f8, u8 = (
        mybir.dt.float32,
        mybir.dt.bfloat16,
        mybir.dt.float8e4,
        mybir.dt.uint8,
    )

    W8_d = nc.dram_tensor("W8r", (128, KT_C0 * D), f8, kind="ExternalInput")
    vT8_d = nc.dram_tensor("vT8r", (128, KT_C0 * B * SV), f8, kind="ExternalInput")
    w16_d = nc.dram_tensor("w16r", (128, 2 * DV), f8, kind="ExternalInput")
    vrep_d = nc.dram_tensor("vrepr", (128, B * DV), bf, kind="ExternalInput")
    pk_d = nc.dram_tensor("pk", (128, PK_BYTES), u8, kind="ExternalInput")
    u_d = nc.dram_tensor("u_out", (ROWS, DV), f32, kind="ExternalOutput")

    KT_C = KT_C0  # 32 k-tiles over the DV contraction (v @ W)

    with tile.TileContext(nc) as tc, ExitStack() as ctx:
        consts = ctx.enter_context(tc.tile_pool(name="consts", bufs=1))
        vreppool = ctx.enter_context(tc.tile_pool(name="vrep", bufs=3))

        ph1_ctx = ExitStack()
        ph1c = ph1_ctx.enter_context(tc.tile_pool(name="ph1c", bufs=1))

        # dummy activation at t=0 so the exp_and_others ACT table loads during
        # the initial DMA wait instead of before the first tanh
        dummy = consts.tile([128, 8], f32)
        nc.vector.memset(dummy, 1.0)
        nc.scalar.activation(dummy, dummy, mybir.ActivationFunctionType.Exp)

        # ---- packed consts: one DMA on gpsimd ------------------------------
        pk_sb = ph1c.tile([128, PK_BYTES], u8)
        nc.gpsimd.dma_start(out=pk_sb, in_=pk_d[:])
        U2v = pk_sb[:, PK_U2 : PK_U2 + 3072].bitcast(f32).rearrange(
            "p (k d) -> p k d", k=3
        )
        hT2v = pk_sb[:, PK_HT : PK_HT + 384].bitcast(f32).rearrange(
            "p (k m) -> p k m", k=3
        )
        I_sb = consts.tile([128, 128], bf)
        nc.vector.tensor_copy(I_sb, pk_sb[:, PK_I : PK_I + 256].bitcast(bf))
        L_sb = consts.tile([128, B, ROWS], bf)
        nc.vector.tensor_copy(
            L_sb,
            pk_sb[:, PK_L : PK_L + 512].bitcast(bf).rearrange(
                "p (b m) -> p b m", b=B
            ),
        )

        # ---- phase-1 weights: W on sync, vT on the scalar HWDGE queue ------
        W_sb = ph1c.tile([128, KT_C, D], f8)
        vT_sb = ph1c.tile([128, KT_C, B * SV], f8)
        NCH = 4
        kch = KT_C // NCH
        for c in range(NCH):
            ks = slice(c * kch, (c + 1) * kch)
            nc.sync.dma_start(
                out=W_sb[:, ks, :], in_=W8_d[:, c * kch * D : (c + 1) * kch * D]
            )
            nc.sync.dma_start(
                out=vT_sb[:, ks, :],
                in_=vT8_d[:, c * kch * B * SV : (c + 1) * kch * B * SV],
            )

        # w16: 2-segment (kt-major HBM) DMAs, chunked by c
        w_sb = consts.tile([128, 2, DV], f8)
        for c in range(4):
            csl = slice(c * 1024, (c + 1) * 1024)
            w_src = w16_d[:]
            w_bc = bass.AP(
                tensor=w_src.tensor,
                offset=w_src.offset + c * 1024,
                ap=[w_src.ap[0], [DV, 2], [1, 1024]],
            )
            nc.sync.dma_start(out=w_sb[:, :, csl], in_=w_bc)

        # ---- v_rep per-iteration loads (pre-replicated in HBM) -------------
        vrep_tiles = {}

        def load_vrep(it, queue):
            wd = WIDTHS[it]
            c0 = COFFS[it]
            t = vreppool.tile([128, B, 1024], bf, tag="vrep", name=f"vrep{it}")
            vrep_tiles[it] = t
            for bb in range(B):
                queue.dma_start(
                    out=t[:, bb, 0:wd],
                    in_=vrep_d[:, bb * DV + c0 : bb * DV + c0 + wd],
                )

        load_vrep(0, nc.gpsimd)
        load_vrep(1, nc.gpsimd)
        load_vrep(2, nc.gpsimd)

        # ---- phase 1: ubias = 8*(U^T h + b), W_v^T ( *8 ), fT = tanh(zz/8) --
        ph1 = ph1_ctx.enter_context(tc.tile_pool(name="ph1", bufs=1, space="PSUM"))

        ub_ps = ph1.tile([128, 2, ROWS], f32)
        for mt in range(2):
            msl = slice(mt * 128, (mt + 1) * 128)
            for kt in range(3):
                ksz = 128 if kt < 2 else 1
                nc.tensor.matmul(
                    ub_ps[:, mt, :],
                    U2v[0:ksz, kt, msl],
                    hT2v[0:ksz, kt, :],
                    start=(kt == 0),
                    stop=(kt == 2),
                )
        ub_sb = ph1c.tile([128, 2, ROWS], f32)
        nc.vector.tensor_copy(ub_sb, ub_ps)

        # Wv via fp8 DoubleRow: 16 MMs per mt, contraction 256 each
        wv_ps = [
            ph1.tile([128, B * SV], f32, tag=f"wv{mt}", name=f"wv_ps{mt}")
            for mt in range(2)
        ]
        for kp in range(KT_C // 2):
            for mt in range(2):
                msl = slice(mt * 128, (mt + 1) * 128)
                nc.tensor.matmul(
                    wv_ps[mt],
                    W_sb[:, 2 * kp : 2 * kp + 2, msl],
                    vT_sb[:, 2 * kp : 2 * kp + 2, :],
                    start=(kp == 0),
                    stop=(kp == KT_C // 2 - 1),
                    perf_mode=DR,
                )

        # zz8[d,(b,h,s)] = 8*Wv^T + 8*ub ; fT = tanh(zz8/8) -> fp8, chunked
        # by b-pairs so the first q matmuls start early.
        zz_sb = ph1c.tile([128, 2, B * SHL * SV], f32)
        fT_sb = consts.tile([128, 2, B * SHL * SV], f8)
        for ch in range(4):  # 2 b's per chunk
            for mt in range(2):
                wv_base = wv_ps[mt][:]
                wv_bc = bass.AP(
                    tensor=wv_base.tensor,
                    offset=wv_base.offset + 2 * ch * SV,
                    ap=[wv_base.ap[0], [SV, 2], [0, SHL], [1, SV]],
                )
                ub_base = ub_sb[:, mt, :]
                ub_bc = bass.AP(
                    tensor=ub_base.tensor,
                    offset=ub_base.offset + 2 * ch * SHL,
                    ap=[ub_base.ap[0], [SHL, 2], [1, SHL], [0, SV]],
                )
                csl = slice(ch * 256, (ch + 1) * 256)
                zz_out = zz_sb[:, mt, csl].rearrange(
                    "p (b h s) -> p b h s", b=2, h=SHL
                )
                nc.vector.tensor_add(zz_out, wv_bc, ub_bc)
            nc.scalar.activation(
                fT_sb[:, :, csl],
                zz_sb[:, :, csl],
                mybir.ActivationFunctionType.Tanh,
                scale=1.0 / W_SCALE,
            )

        ph1_ctx.close()

        # ---- phase 2: q -> e -> S -> R -> ev*r -> u ------------------------
        epool = ctx.enter_context(tc.tile_pool(name="epool", bufs=2))
        evpool = ctx.enter_context(tc.tile_pool(name="evpool", bufs=2))
        gvpool = ctx.enter_context(tc.tile_pool(name="gvpool", bufs=2))
        usbpool = ctx.enter_context(tc.tile_pool(name="usbpool", bufs=3))
        r32pool = ctx.enter_context(tc.tile_pool(name="r32pool", bufs=2))
        rpool = ctx.enter_context(tc.tile_pool(name="rpool", bufs=2))
        qpool = ctx.enter_context(tc.tile_pool(name="qpool", bufs=2, space="PSUM"))
        spool = ctx.enter_context(tc.tile_pool(name="spool", bufs=1, space="PSUM"))
        upool = ctx.enter_context(tc.tile_pool(name="upool", bufs=2, space="PSUM"))

        pendings = []  # deferred u-blocks (depth 1)

        def emit_u_block(gv_t, it):
            wd = WIDTHS[it]
            c0 = COFFS[it]
            nj = max(1, wd // 256)
            nwj = wd // nj
            u_ps = upool.tile([128, nwj], f32, tag="u", name=f"u_ps_{it}")
            for bb in range(B):
                for j in range(nj):
                    nc.tensor.matmul(
                        u_ps[32 * j : 32 * (j + 1), :],
                        L_sb[:, bb, :],
                        gv_t[:, bb, j * nwj : (j + 1) * nwj],
                        start=(bb == 0),
                        stop=(bb == B - 1),
                        tile_position=(0, 32 * j),
                        skip_group_check=True,
                    )
            u_sb = usbpool.tile([ROWS, wd], f32, tag="u_sb", name=f"u_sb_{it}")
            for j in range(nj):
                src = u_ps[32 * j : 32 * (j + 1), :]
                dst = u_sb[:, j * nwj : (j + 1) * nwj]
                if j % 2 == 0:
                    nc.scalar.copy(dst, src)
                else:
                    nc.vector.tensor_copy(dst, src)
            nc.sync.dma_start(out=u_d[:, c0 : c0 + wd], in_=u_sb)

        pending_rgv = None  # (it_prev, r32 tile, ev tile)

        def emit_cast_gv(pit, r32, pev):
            pwd = WIDTHS[pit]
            r_bf = rpool.tile([128, 1024], bf, tag="r", name=f"r_{pit}")
            nc.scalar.copy(r_bf[:, 0:pwd], r32[:, 0:pwd])
            r_base = r_bf[:, 0:pwd]
            gv_t = gvpool.tile([128, B, 1024], bf, tag="gv", name=f"gv_{pit}")
            for bg in range(2):
                r_bc = bass.AP(
                    tensor=r_base.tensor,
                    offset=r_base.offset,
                    ap=[r_base.ap[0], [0, 4], [1, pwd]],
                )
                ev_sl = bass.AP(
                    tensor=pev[:].tensor,
                    offset=pev[:].offset + 4 * bg * 1024,
                    ap=[pev[:].ap[0], [1024, 4], [1, pwd]],
                )
                gv_sl = bass.AP(
                    tensor=gv_t[:].tensor,
                    offset=gv_t[:].offset + 4 * bg * 1024,
                    ap=[gv_t[:].ap[0], [1024, 4], [1, pwd]],
                )
                nc.vector.tensor_mul(gv_sl, ev_sl, r_bc)
            return gv_t

        for it in range(NIT):
            wd = WIDTHS[it]
            c0 = COFFS[it]
            v_rep = vrep_tiles[it]
            e_all = epool.tile([128, B, 1024], bf, tag="e", name=f"e_{it}")
            ev_t = evpool.tile([128, B, 1024], bf, tag="ev", name=f"ev_{it}")
            nhalf = wd // 512
            for bb in range(B):
                q_ps = qpool.tile([128, 1024], f32, tag="q", name=f"q_{it}_{bb}")
                for half in range(nhalf):
                    nsl = slice(c0 + half * 512, c0 + (half + 1) * 512)
                    nc.tensor.matmul(
                        q_ps[:, half * 512 : (half + 1) * 512],
                        fT_sb[:, :, bb * 128 : (bb + 1) * 128],
                        w_sb[:, :, nsl],
                        start=True,
                        stop=True,
                        perf_mode=DR,
                        skip_group_check=True,
                    )
                nc.scalar.activation(
                    e_all[:, bb, 0:wd],
                    q_ps[:, 0:wd],
                    mybir.ActivationFunctionType.Exp,
                    scale=1.0 / SMALL_SCALE,
                )
                if bb % 2 == 1:
                    # ev = e*v is not r-gated: runs on DVE during the exp phase
                    bsl2 = slice(bb - 1, bb + 1)
                    nc.vector.tensor_mul(
                        ev_t[:, bsl2, 0:wd].rearrange("p b w -> p (b w)")
                        if wd == 1024
                        else bass.AP(
                            tensor=ev_t[:].tensor,
                            offset=ev_t[:].offset + (bb - 1) * 1024,
                            ap=[ev_t[:].ap[0], [1024, 2], [1, wd]],
                        ),
                        bass.AP(
                            tensor=e_all[:].tensor,
                            offset=e_all[:].offset + (bb - 1) * 1024,
                            ap=[e_all[:].ap[0], [1024, 2], [1, wd]],
                        ),
                        bass.AP(
                            tensor=v_rep[:].tensor,
                            offset=v_rep[:].offset + (bb - 1) * 1024,
                            ap=[v_rep[:].ap[0], [1024, 2], [1, wd]],
                        ),
                    )
                if bb == 2 and pending_rgv is not None:
                    pit, pr32, pev = pending_rgv
                    pending_rgv = None
                    gv_prev = emit_cast_gv(pit, pr32, pev)
                    pendings.append((gv_prev, pit))
                    if pit + 3 < NIT:
                        load_vrep(pit + 3, nc.sync if pit == 0 else nc.gpsimd)

            last = it == NIT - 1
            # deferred u-block now: PE work while the exps/S/recip run
            while pendings:
                emit_u_block(*pendings.pop(0))

            s_ps = spool.tile([128, 1024], f32, tag="s_ps", name=f"s_ps{it}")
            for bb in range(B):
                for half in range(nhalf):
                    hsl = slice(half * 512, (half + 1) * 512)
                    nc.tensor.matmul(
                        s_ps[:, hsl],
                        I_sb,
                        e_all[:, bb, hsl],
                        start=(bb == 0),
                        stop=(bb == B - 1),
                        skip_group_check=True,
                    )

            if not last:
                r32 = r32pool.tile([128, 1024], f32, tag="r32", name=f"r32_{it}")
                nc.vector.reciprocal_approx_fast(r32[:, 0:wd], s_ps[:, 0:wd])
                pending_rgv = (it, r32, ev_t)
            else:
                # tail: small all-DVE chunks; serial chain is short
                for cc, (cs, cw) in enumerate([(0, 256), (256, 128), (384, 128)]):
                    r32c = r32pool.tile(
                        [128, 256], f32, tag="r32t", name=f"r32t{cc}"
                    )
                    nc.vector.reciprocal_approx_fast(
                        r32c[:, 0:cw], s_ps[:, cs : cs + cw]
                    )
                    r_bfc = rpool.tile([128, 256], bf, tag="rt", name=f"rt{cc}")
                    nc.vector.tensor_copy(r_bfc[:, 0:cw], r32c[:, 0:cw])
                    r_base = r_bfc[:, 0:cw]
                    r_bc = bass.AP(
                        tensor=r_base.tensor,
                        offset=r_base.offset,
                        ap=[r_base.ap[0], [0, B], [1, cw]],
                    )
                    ev_sl = bass.AP(
                        tensor=ev_t[:].tensor,
                        offset=ev_t[:].offset + cs,
                        ap=[ev_t[:].ap[0], [1024, B], [1, cw]],
                    )
                    gv_t = gvpool.tile(
                        [128, B, 256], bf, tag="gvt", name=f"gvt{cc}"
                    )
                    nc.vector.tensor_mul(gv_t[:, :, 0:cw], ev_sl, r_bc)
                    nj = cw // 128
                    u_ps = upool.tile([128, 128], f32, tag="u", name=f"upst{cc}")
                    for bb in range(B):
                        for j in range(nj):
                            nc.tensor.matmul(
                                u_ps[32 * j : 32 * (j + 1), :],
                                L_sb[:, bb, :],
                                gv_t[:, bb, j * 128 : (j + 1) * 128],
                                start=(bb == 0),
                                stop=(bb == B - 1),
                                tile_position=(0, 32 * j),
                                skip_group_check=True,
                            )
                    u_sb = usbpool.tile(
                        [ROWS, 256], f32, tag="u_sbt", name=f"usbt{cc}"
                    )
                    for j in range(nj):
                        nc.scalar.copy(
                            u_sb[:, j * 128 : (j + 1) * 128],
                            u_ps[32 * j : 32 * (j + 1), :],
                        )
                    nc.sync.dma_start(
                        out=u_d[:, c0 + cs : c0 + cs + cw], in_=u_sb[:, 0:cw]
                    )

    nc.compile()
    return nc


def _install_profile_hook():
    """The image's antenv lacks axon_hooks; inject it and register the
    ctypes NTFF hook from trn_agent_boot so trace=True works under axon."""
    import types

    try:
        from antenv.axon_hooks import get_axon_ntff_profile_hook  # noqa: F401

        return
    except ImportError:
        pass
    import antenv

    mod = types.ModuleType("antenv.axon_hooks")
    holder = {"hook": None}
    mod.set_axon_ntff_profile_hook = lambda h: holder.__setitem__("hook", h)
    mod.get_axon_ntff_profile_hook = lambda: holder["hook"]
    sys.modules["antenv.axon_hooks"] = mod
    antenv.axon_hooks = mod
    try:
        if "/root/.axon_site" not in sys.path:
            sys.path.insert(0, "/root/.axon_site")
        from trn_agent_boot.trn_boot import _ntff_profile_via_ctypes

        mod.set_axon_ntff_profile_hook(
            _ntff_profile_via_ctypes("/opt/axon/libaxon_pjrt.so")
        )
    except Exception as ex:  # degrade: tracing skipped, run still works
        print("profile hook install failed:", ex)
    # artifact upload needs bucket creds this container doesn't have
    import concourse.bass_utils as bu

    bu.upload_artifacts = lambda tmpdir: "local://" + tmpdir


_NC_CACHE = {}


def _get_nc():
    if "nc" not in _NC_CACHE:
        _NC_CACHE["nc"] = build_nc()
    return _NC_CACHE["nc"]


def make_inputs(h, v, W, U, b, w):
    """Host-side prep: shared tensors + per-core in_maps."""
    W8 = (W * W_SCALE).astype(F8)
    W8r = np.ascontiguousarray(
        W8.reshape(KT_C0, 128, D).transpose(1, 0, 2).reshape(128, KT_C0 * D)
    )
    vT8 = np.ascontiguousarray(v.transpose(2, 0, 1).reshape(DV, B * SV)).astype(F8)
    vT8r = np.ascontiguousarray(
        vT8.reshape(KT_C0, 128, B * SV).transpose(1, 0, 2).reshape(128, -1)
    )
    w16 = (w * SMALL_SCALE).astype(F8)
    w16r = np.ascontiguousarray(
        w16.reshape(2, 128, DV).transpose(1, 0, 2).reshape(128, 2 * DV)
    )
    # v_rep pre-replicated over the 4 local h positions: row p=(h,s)
    vrep = np.ascontiguousarray(
        np.tile(v.transpose(1, 0, 2).reshape(SV, B * DV).astype(BF16), (SHL, 1))
    )
    U8 = np.ascontiguousarray(W_SCALE * U).astype(F32)
    b8 = np.ascontiguousarray(W_SCALE * b).astype(F32)
    Ieye = np.eye(128, dtype=BF16)
    Lsum = np.zeros((B, 128, ROWS), dtype=BF16)
    for bb in range(B):
        for hh in range(SHL):
            for ss in range(SV):
                Lsum[bb, hh * SV + ss, bb * SHL + hh] = 1
    Lr = np.ascontiguousarray(Lsum.transpose(1, 0, 2).reshape(128, B * ROWS))

    in_maps = []
    for core in range(NCORES):
        hsl = h[:, core * SHL : (core + 1) * SHL, :]  # (B, SHL, DH)
        hT = np.ascontiguousarray(hsl.transpose(2, 0, 1).reshape(DH, ROWS)).astype(
            F32
        )
        pk = np.zeros((128, PK_BYTES), dtype=np.uint8)
        pk[:, 0:1024] = U8[0:128].view(np.uint8)
        pk[:, 1024:2048] = U8[128:256].view(np.uint8)
        pk[0, 2048:3072] = b8.view(np.uint8)
        pk[:, 3072:3200] = hT[0:128].view(np.uint8)
        pk[:, 3200:3328] = hT[128:256].view(np.uint8)
        pk[0, 3328:3456] = np.ones(ROWS, F32).view(np.uint8)
        pk[:, PK_I : PK_I + 256] = Ieye.view(np.uint8)
        pk[:, PK_L : PK_L + 512] = Lr.view(np.uint8)
        in_maps.append(
            {
                "W8r": W8r,
                "vT8r": vT8r,
                "w16r": w16r,
                "vrepr": vrep,
                "pk": pk,
            }
        )
    return in_maps


def gather_output(results):
    u_full = np.empty((B, SH, DV), dtype=F32)
    for core, res in enumerate(results):
        u_full[:, core * SHL : (core + 1) * SHL, :] = res["u_out"].reshape(
            B, SHL, DV
        )
    return u_full


def kernel(h, v, W, U, b, w, trace: bool = False):
    if trace:
        _install_profile_hook()
    nc = _get_nc()
    in_maps = make_inputs(
        np.asarray(h, F32),
        np.asarray(v, F32),
        np.asarray(W, F32),
        np.asarray(U, F32),
        np.asarray(b, F32),
        np.asarray(w, F32),
    )
    out = run_bass_kernel_spmd(nc, in_maps, core_ids=list(range(NCORES)), trace=trace)
    res = gather_output(out.results)
    if trace:
        kernel.last_exec_time_ns = out.exec_time_ns
        kernel.last_trace = out.instructions_and_trace
    return res


# revision 18
# speedup vs baseline: 1.2119x; 1.2119x over previous
"""Trainium2 Bass kernel for nn_Attention_Layer (B=8, SH=SV=32, DH=D=256, DV=4096).

Math (see reference):
    U_h = h @ U                  (B,SH,D)
    W_v = v @ W                  (B,SV,D)
    f   = tanh(W_v + U_h + b)    (B,SH,SV,D)
    q   = f @ w                  (B,SH,SV,DV)
    e   = exp(q); S = sum_b e; beta = e/S
    u   = sum_sv beta * v        (B,SH,DV)

Sharding: over SH (4 h-positions per core), no collectives.

v4 design:
  - fp8e4m3 DoubleRow for the Wv and q matmuls (W*8, vT, f, w*16 in fp8;
    scale folded into tanh/exp). rel-err ~8.6e-3 vs the 2e-2 gate.
  - 2 DMA queues: W/w/vrep3 on sync (SP); packed consts, vT and the other
    v_rep tiles on gpsimd; all layouts host-rearranged so every DMA is
    contiguous per partition.
  - All small consts (U2/hT2/I/Lsum) ship as ONE packed uint8 DMA, consumed
    through bitcast views.
  - ev = e*v runs per-b on DVE during the exp phase; only gv = ev*r is
    r-gated. u-block deferral depth 1.
  - Iteration widths [1024,1024,1024,512,512]: the last iteration is small
    and its recip/cast/gv/u chain runs in 256-wide all-DVE chunks, so the
    post-last-exp serial tail is short.
"""

import sys

sys.path.insert(0, "/opt/trn_rl_repo")

from contextlib import ExitStack

import ml_dtypes
import numpy as np

import concourse.bass as bass
import concourse.mybir as mybir
import concourse.tile as tile
from concourse import bacc
from concourse.bass_utils import run_bass_kernel_spmd

BF16 = ml_dtypes.bfloat16
F8 = ml_dtypes.float8_e4m3
F32 = np.float32

B, SH, SV, DH, DV, D = 8, 32, 32, 256, 4096, 256
KT_C0 = DV // 128
NCORES = 8
SHL = SH // NCORES  # 4 h-positions per core
ROWS = B * SHL  # 32 output rows per core, index = b*SHL + h
WIDTHS = [1024, 1024, 1024, 512, 512]
COFFS = [0, 1024, 2048, 3072, 3584]
NIT = len(WIDTHS)
W_SCALE = 8.0
SMALL_SCALE = 16.0  # w scale

# packed-consts byte offsets (per partition)
PK_U2 = 0  # 3*256 f32 = 3072B  (kt0, kt1, bias-row-on-p0)
PK_HT = 3072  # 3*32 f32 = 384B
PK_I = 3456  # 128 bf16 = 256B
PK_L = 3712  # 8*32 bf16 = 512B
PK_BYTES = 4224

DR = mybir.MatmulPerfMode.DoubleRow


def build_nc(debug: bool = False):
    nc = bacc.Bacc("TRN2", target_bir_lowering=False, debug=debug)
    f32, bf, f8, u8 = (
        mybir.dt.float32,
        mybir.dt.bfloat16,
        mybir.dt.float8e4,
        mybir.dt.uint8,
    )

    W8_d = nc.dram_tensor("W8r", (128, KT_C0 * D), f8, kind="ExternalInput")
    vT8_d = nc.dram_tensor("vT8r", (128, KT_C0 * B * SV), f8, kind="ExternalInput")
    w16_d = nc.dram_tensor("w16r", (128, 2 * DV), f8, kind="ExternalInput")
    vrep_d = nc.dram_tensor("vrepr", (128, B * DV), bf, kind="ExternalInput")
    pk_d = nc.dram_tensor("pk", (128, PK_BYTES), u8, kind="ExternalInput")
    u_d = nc.dram_tensor("u_out", (ROWS, DV), f32, kind="ExternalOutput")

    KT_C = KT_C0  # 32 k-tiles over the DV contraction (v @ W)

    with tile.TileContext(nc) as tc, ExitStack() as ctx:
        consts = ctx.enter_context(tc.tile_pool(name="consts", bufs=1))
        vreppool = ctx.enter_context(tc.tile_pool(name="vrep", bufs=3))

        ph1_ctx = ExitStack()
        ph1c = ph1_ctx.enter_context(tc.tile_pool(name="ph1c", bufs=1))

        # dummy activation at t=0 so the exp_and_others ACT table loads during
        # the initial DMA wait instead of before the first tanh
        dummy = consts.tile([128, 8], f32)
        nc.vector.memset(dummy, 1.0)
        nc.scalar.activation(dummy, dummy, mybir.ActivationFunctionType.Exp)

        # ---- phase-1 weights: W on sync, vT on the scalar HWDGE queue ------
        W_sb = ph1c.tile([128, KT_C, D], f8)
        vT_sb = ph1c.tile([128, KT_C, B * SV], f8)
        NCH = 4
        kch = KT_C // NCH
        for c in range(NCH):
            ks = slice(c * kch, (c + 1) * kch)
            nc.gpsimd.dma_start(
                out=vT_sb[:, ks, :],
                in_=vT8_d[:, c * kch * B * SV : (c + 1) * kch * B * SV],
            )
            nc.sync.dma_start(
                out=W_sb[:, ks, :], in_=W8_d[:, c * kch * D : (c + 1) * kch * D]
            )

        # w16: 2-segment (kt-major HBM) DMAs, chunked by c
        w_sb = consts.tile([128, 2, DV], f8)
        for c in range(4):
            csl = slice(c * 1024, (c + 1) * 1024)
            w_src = w16_d[:]
            w_bc = bass.AP(
                tensor=w_src.tensor,
                offset=w_src.offset + c * 1024,
                ap=[w_src.ap[0], [DV, 2], [1, 1024]],
            )
            nc.sync.dma_start(out=w_sb[:, :, csl], in_=w_bc)

        # ---- packed consts: one DMA on gpsimd ------------------------------
        pk_sb = ph1c.tile([128, PK_BYTES], u8)
        nc.gpsimd.dma_start(out=pk_sb, in_=pk_d[:])
        U2v = pk_sb[:, PK_U2 : PK_U2 + 3072].bitcast(f32).rearrange(
            "p (k d) -> p k d", k=3
        )
        hT2v = pk_sb[:, PK_HT : PK_HT + 384].bitcast(f32).rearrange(
            "p (k m) -> p k m", k=3
        )
        I_sb = consts.tile([128, 128], bf)
        nc.vector.tensor_copy(I_sb, pk_sb[:, PK_I : PK_I + 256].bitcast(bf))
        L_sb = consts.tile([128, B, ROWS], bf)
        nc.vector.tensor_copy(
            L_sb,
            pk_sb[:, PK_L : PK_L + 512].bitcast(bf).rearrange(
                "p (b m) -> p b m", b=B
            ),
        )

        # ---- v_rep per-iteration loads (pre-replicated in HBM) -------------
        vrep_tiles = {}

        def load_vrep(it, queue):
            wd = WIDTHS[it]
            c0 = COFFS[it]
            t = vreppool.tile([128, B, 1024], bf, tag="vrep", name=f"vrep{it}")
            vrep_tiles[it] = t
            for bb in range(B):
                queue.dma_start(
                    out=t[:, bb, 0:wd],
                    in_=vrep_d[:, bb * DV + c0 : bb * DV + c0 + wd],
                )

        load_vrep(0, nc.gpsimd)
        load_vrep(1, nc.gpsimd)

        # ---- phase 1: ubias = 8*(U^T h + b), W_v^T ( *8 ), fT = tanh(zz/8) --
        ph1 = ph1_ctx.enter_context(tc.tile_pool(name="ph1", bufs=1, space="PSUM"))

        # Wv via fp8 DoubleRow: 16 MMs per mt, contraction 256 each
        wv_ps = [
            ph1.tile([128, B * SV], f32, tag=f"wv{mt}", name=f"wv_ps{mt}")
            for mt in range(2)
        ]
        for kp in range(KT_C // 2):
            for mt in range(2):
                msl = slice(mt * 128, (mt + 1) * 128)
                nc.tensor.matmul(
                    wv_ps[mt],
                    W_sb[:, 2 * kp : 2 * kp + 2, msl],
                    vT_sb[:, 2 * kp : 2 * kp + 2, :],
                    start=(kp == 0),
                    stop=(kp == KT_C // 2 - 1),
                    perf_mode=DR,
                )

        ub_ps = ph1.tile([128, 2, ROWS], f32)
        for mt in range(2):
            msl = slice(mt * 128, (mt + 1) * 128)
            for kt in range(3):
                ksz = 128 if kt < 2 else 1
                nc.tensor.matmul(
                    ub_ps[:, mt, :],
                    U2v[0:ksz, kt, msl],
                    hT2v[0:ksz, kt, :],
                    start=(kt == 0),
                    stop=(kt == 2),
                )
        ub_sb = ph1c.tile([128, 2, ROWS], f32)
        nc.vector.tensor_copy(ub_sb, ub_ps)

        # zz8[d,(b,h,s)] = 8*Wv^T + 8*ub ; fT = tanh(zz8/8) -> fp8, chunked
        # by b-pairs so the first q matmuls start early.
        zz_sb = ph1c.tile([128, 2, B * SHL * SV], f32)
        fT_sb = consts.tile([128, 2, B * SHL * SV], f8)
        for ch in range(4):  # 2 b's per chunk
            for mt in range(2):
                wv_base = wv_ps[mt][:]
                wv_bc = bass.AP(
                    tensor=wv_base.tensor,
                    offset=wv_base.offset + 2 * ch * SV,
                    ap=[wv_base.ap[0], [SV, 2], [0, SHL], [1, SV]],
                )
                ub_base = ub_sb[:, mt, :]
                ub_bc = bass.AP(
                    tensor=ub_base.tensor,
                    offset=ub_base.offset + 2 * ch * SHL,
                    ap=[ub_base.ap[0], [SHL, 2], [1, SHL], [0, SV]],
                )
                csl = slice(ch * 256, (ch + 1) * 256)
                zz_out = zz_sb[:, mt, csl].rearrange(
                    "p (b h s) -> p b h s", b=2, h=SHL
                )
                nc.vector.tensor_add(zz_out, wv_bc, ub_bc)
            nc.scalar.activation(
                fT_sb[:, :, csl],
                zz_sb[:, :, csl],
                mybir.ActivationFunctionType.Tanh,
                scale=1.0 / W_SCALE,
            )

        ph1_ctx.close()

        # ---- phase 2: q -> e -> S -> R -> ev*r -> u ------------------------
        epool = ctx.enter_context(tc.tile_pool(name="epool", bufs=2))
        evpool = ctx.enter_context(tc.tile_pool(name="evpool", bufs=2))
        gvpool = ctx.enter_context(tc.tile_pool(name="gvpool", bufs=2))
        usbpool = ctx.enter_context(tc.tile_pool(name="usbpool", bufs=3))
        r32pool = ctx.enter_context(tc.tile_pool(name="r32pool", bufs=2))
        rpool = ctx.enter_context(tc.tile_pool(name="rpool", bufs=2))
        qpool = ctx.enter_context(tc.tile_pool(name="qpool", bufs=2, space="PSUM"))
        spool = ctx.enter_context(tc.tile_pool(name="spool", bufs=1, space="PSUM"))
        upool = ctx.enter_context(tc.tile_pool(name="upool", bufs=2, space="PSUM"))

        pendings = []  # deferred u-blocks (depth 1)

        def emit_u_block(gv_t, it):
            wd = WIDTHS[it]
            c0 = COFFS[it]
            nj = max(1, wd // 256)
            nwj = wd // nj
            u_ps = upool.tile([128, nwj], f32, tag="u", name=f"u_ps_{it}")
            for bb in range(B):
                for j in range(nj):
                    nc.tensor.matmul(
                        u_ps[32 * j : 32 * (j + 1), :],
                        L_sb[:, bb, :],
                        gv_t[:, bb, j * nwj : (j + 1) * nwj],
                        start=(bb == 0),
                        stop=(bb == B - 1),
                        tile_position=(0, 32 * j),
                        skip_group_check=True,
                    )
            u_sb = usbpool.tile([ROWS, wd], f32, tag="u_sb", name=f"u_sb_{it}")
            for j in range(nj):
                src = u_ps[32 * j : 32 * (j + 1), :]
                dst = u_sb[:, j * nwj : (j + 1) * nwj]
                if j % 2 == 0:
                    nc.scalar.copy(dst, src)
                else:
                    nc.vector.tensor_copy(dst, src)
            nc.sync.dma_start(out=u_d[:, c0 : c0 + wd], in_=u_sb)

        for it in range(NIT):
            wd = WIDTHS[it]
            c0 = COFFS[it]
            v_rep = vrep_tiles[it]
            e_all = epool.tile([128, B, 1024], bf, tag="e", name=f"e_{it}")
            ev_t = evpool.tile([128, B, 1024], bf, tag="ev", name=f"ev_{it}")
            nhalf = wd // 512
            for bb in range(B):
                q_ps = qpool.tile([128, 1024], f32, tag="q", name=f"q_{it}_{bb}")
                for half in range(nhalf):
                    nsl = slice(c0 + half * 512, c0 + (half + 1) * 512)
                    nc.tensor.matmul(
                        q_ps[:, half * 512 : (half + 1) * 512],
                        fT_sb[:, :, bb * 128 : (bb + 1) * 128],
                        w_sb[:, :, nsl],
                        start=True,
                        stop=True,
                        perf_mode=DR,
                        skip_group_check=True,
                    )
                nc.scalar.activation(
                    e_all[:, bb, 0:wd],
                    q_ps[:, 0:wd],
                    mybir.ActivationFunctionType.Exp,
                    scale=1.0 / SMALL_SCALE,
                )
                # ev = e*v is not r-gated: runs on DVE during the exp phase
                nc.vector.tensor_mul(
                    ev_t[:, bb, 0:wd], e_all[:, bb, 0:wd], v_rep[:, bb, 0:wd]
                )

            last = it == NIT - 1
            # deferred u-block now: PE work while the exps/S/recip run
            while pendings:
                emit_u_block(*pendings.pop(0))

            s_ps = spool.tile([128, 1024], f32, tag="s_ps", name=f"s_ps{it}")
            for bb in range(B):
                for half in range(nhalf):
                    hsl = slice(half * 512, (half + 1) * 512)
                    nc.tensor.matmul(
                        s_ps[:, hsl],
                        I_sb,
                        e_all[:, bb, hsl],
                        start=(bb == 0),
                        stop=(bb == B - 1),
                        skip_group_check=True,
                    )

            if not last:
                r32 = r32pool.tile([128, 1024], f32, tag="r32", name=f"r32_{it}")
                nc.vector.reciprocal_approx_fast(r32[:, 0:wd], s_ps[:, 0:wd])
                r_bf = rpool.tile([128, 1024], bf, tag="r", name=f"r_{it}")
                nc.scalar.copy(r_bf[:, 0:wd], r32[:, 0:wd])

                r_base = r_bf[:, 0:wd]
                gv_t = gvpool.tile([128, B, 1024], bf, tag="gv", name=f"gv_{it}")
                for bg in range(2):
                    r_bc = bass.AP(
                        tensor=r_base.tensor,
                        offset=r_base.offset,
                        ap=[r_base.ap[0], [0, 4], [1, wd]],
                    )
                    ev_sl = bass.AP(
                        tensor=ev_t[:].tensor,
                        offset=ev_t[:].offset + 4 * bg * 1024,
                        ap=[ev_t[:].ap[0], [1024, 4], [1, wd]],
                    )
                    gv_sl = bass.AP(
                        tensor=gv_t[:].tensor,
                        offset=gv_t[:].offset + 4 * bg * 1024,
                        ap=[gv_t[:].ap[0], [1024, 4], [1, wd]],
                    )
                    nc.vector.tensor_mul(gv_sl, ev_sl, r_bc)

                pendings.append((gv_t, it))
                # prefetch later v_reps after this iteration's gv (ring WAR)
                if it == 0:
                    load_vrep(2, nc.gpsimd)
                    load_vrep(3, nc.sync)
                elif it == 1:
                    load_vrep(4, nc.gpsimd)
            else:
                # tail: 256-wide all-DVE chunks; serial chain is short
                for cc in range(wd // 256):
                    cs = cc * 256
                    r32c = r32pool.tile(
                        [128, 256], f32, tag="r32t", name=f"r32t{cc}"
                    )
                    nc.vector.reciprocal_approx_fast(r32c, s_ps[:, cs : cs + 256])
                    r_bfc = rpool.tile([128, 256], bf, tag="rt", name=f"rt{cc}")
                    nc.vector.tensor_copy(r_bfc, r32c)
                    r_base = r_bfc[:]
                    r_bc = bass.AP(
                        tensor=r_base.tensor,
                        offset=r_base.offset,
                        ap=[r_base.ap[0], [0, B], [1, 256]],
                    )
                    ev_sl = bass.AP(
                        tensor=ev_t[:].tensor,
                        offset=ev_t[:].offset + cs,
                        ap=[ev_t[:].ap[0], [1024, B], [1, 256]],
                    )
                    gv_t = gvpool.tile(
                        [128, B, 256], bf, tag="gvt", name=f"gvt{cc}"
                    )
                    nc.vector.tensor_mul(gv_t, ev_sl, r_bc)
                    u_ps = upool.tile([128, 128], f32, tag="u", name=f"upst{cc}")
                    for bb in range(B):
                        for j in range(2):
                            nc.tensor.matmul(
                                u_ps[32 * j : 32 * (j + 1), :],
                                L_sb[:, bb, :],
                                gv_t[:, bb, j * 128 : (j + 1) * 128],
                                start=(bb == 0),
                                stop=(bb == B - 1),
                                tile_position=(0, 32 * j),
                                skip_group_check=True,
                            )
                    u_sb = usbpool.tile(
                        [ROWS, 256], f32, tag="u_sbt", name=f"usbt{cc}"
                    )
                    nc.scalar.copy(u_sb[:, 0:128], u_ps[0:32, :])
                    nc.vector.tensor_copy(u_sb[:, 128:256], u_ps[32:64, :])
                    nc.sync.dma_start(
                        out=u_d[:, c0 + cs : c0 + cs + 256], in_=u_sb
                    )

    nc.compile()
    return nc


def _install_profile_hook():
    """The image's antenv lacks axon_hooks; inject it and register the
    ctypes NTFF hook from trn_agent_boot so trace=True works under axon."""
    import types

    try:
        from antenv.axon_hooks import get_axon_ntff_profile_hook  # noqa: F401

        return
    except ImportError:
        pass
    import antenv

    mod = types.ModuleType("antenv.axon_hooks")
    holder = {"hook": None}
    mod.set_axon_ntff_profile_hook = lambda h: holder.__setitem__("hook", h)
    mod.get_axon_ntff_profile_hook = lambda: holder["hook"]
    sys.modules["antenv.axon_hooks"] = mod
    antenv.axon_hooks = mod
    try:
        if "/root/.axon_site" not in sys.path:
            sys.path.insert(0, "/root/.axon_site")
        from trn_agent_boot.trn_boot import _ntff_profile_via_ctypes

        mod.set_axon_ntff_profile_hook(
            _ntff_profile_via_ctypes("/opt/axon/libaxon_pjrt.so")
        )
    except Exception as ex:  # degrade: tracing skipped, run still works
        print("profile hook install failed:", ex)
    # artifact upload needs bucket creds this container doesn't have
    import concourse.bass_utils as bu

    bu.upload_artifacts = lambda tmpdir: "local://" + tmpdir


_NC_CACHE = {}


def _get_nc():
    if "nc" not in _NC_CACHE:
        _NC_CACHE["nc"] = build_nc()
    return _NC_CACHE["nc"]


def make_inputs(h, v, W, U, b, w):
    """Host-side prep: shared tensors + per-core in_maps."""
    W8 = (W * W_SCALE).astype(F8)
    W8r = np.ascontiguousarray(
        W8.reshape(KT_C0, 128, D).transpose(1, 0, 2).reshape(128, KT_C0 * D)
    )
    vT8 = np.ascontiguousarray(v.transpose(2, 0, 1).reshape(DV, B * SV)).astype(F8)
    vT8r = np.ascontiguousarray(
        vT8.reshape(KT_C0, 128, B * SV).transpose(1, 0, 2).reshape(128, -1)
    )
    w16 = (w * SMALL_SCALE).astype(F8)
    w16r = np.ascontiguousarray(
        w16.reshape(2, 128, DV).transpose(1, 0, 2).reshape(128, 2 * DV)
    )
    # v_rep pre-replicated over the 4 local h positions: row p=(h,s)
    vrep = np.ascontiguousarray(
        np.tile(v.transpose(1, 0, 2).reshape(SV, B * DV).astype(BF16), (SHL, 1))
    )
    U8 = np.ascontiguousarray(W_SCALE * U).astype(F32)
    b8 = np.ascontiguousarray(W_SCALE * b).astype(F32)
    Ieye = np.eye(128, dtype=BF16)
    Lsum = np.zeros((B, 128, ROWS), dtype=BF16)
    for bb in range(B):
        for hh in range(SHL):
            for ss in range(SV):
                Lsum[bb, hh * SV + ss, bb * SHL + hh] = 1
    Lr = np.ascontiguousarray(Lsum.transpose(1, 0, 2).reshape(128, B * ROWS))

    in_maps = []
    for core in range(NCORES):
        hsl = h[:, core * SHL : (core + 1) * SHL, :]  # (B, SHL, DH)
        hT = np.ascontiguousarray(hsl.transpose(2, 0, 1).reshape(DH, ROWS)).astype(
            F32
        )
        pk = np.zeros((128, PK_BYTES), dtype=np.uint8)
        pk[:, 0:1024] = U8[0:128].view(np.uint8)
        pk[:, 1024:2048] = U8[128:256].view(np.uint8)
        pk[0, 2048:3072] = b8.view(np.uint8)
        pk[:, 3072:3200] = hT[0:128].view(np.uint8)
        pk[:, 3200:3328] = hT[128:256].view(np.uint8)
        pk[0, 3328:3456] = np.ones(ROWS, F32).view(np.uint8)
        pk[:, PK_I : PK_I + 256] = Ieye.view(np.uint8)
        pk[:, PK_L : PK_L + 512] = Lr.view(np.uint8)
        in_maps.append(
            {
                "W8r": W8r,
                "vT8r": vT8r,
                "w16r": w16r,
                "vrepr": vrep,
                "pk": pk,
            }
        )
    return in_maps


def gather_output(results):
    u_full = np.empty((B, SH, DV), dtype=F32)
    for core, res in enumerate(results):
        u_full[:, core * SHL : (core + 1) * SHL, :] = res["u_out"].reshape(
            B, SHL, DV
        )
    return u_full


def kernel(h, v, W, U, b, w, trace: bool = False):
    if trace:
        _install_profile_hook()
    nc = _get_nc()
    in_maps = make_inputs(
        np.asarray(h, F32),
        np.asarray(v, F32),
        np.asarray(W, F32),
        np.asarray(U, F32),
        np.asarray(b, F32),
        np.asarray(w, F32),
    )
    out = run_bass_kernel_spmd(nc, in_maps, core_ids=list(range(NCORES)), trace=trace)
    res = gather_output(out.results)
    if trace:
        kernel.last_exec_time_ns = out.exec_time_ns
        kernel.last_trace = out.instructions_and_trace
    return res
